# revision 8
# baseline (speedup 1.0000x reference)
"""GCMC graph-conv kernel for Trainium2, 8-core SPMD.

out = ci * segment_sum((weight[node_ids] * cj)[src_idx], dst_idx)

Strategy (edge sharding by dst range):
  - core k owns dst rows [k*12500, (k+1)*12500); its edges are host-partitioned
    and sorted by (dst_tile, src_chunk, src)
  - messages are fetched straight from the weight table with SWDGE dma_gather
    (int16 indices -> the table is addressed in 4 chunks of 25000 rows)
  - the cj scale is folded into the segment-sum matmul: for each block of 128
    gathered rows, DVE builds onehot[p, d] = (iota[d] == dst_local[p]) * cj[p]
    in one tensor_scalar op, and TensorE accumulates psum[d, :] += onehot.T @ w
  - flush: ACT copies psum*ci to SBUF, HWDGE DMAs the dst tile out

All multiplies stay on device (cj lands in the one-hot, applied by the PE);
the host only partitions/sorts edges and stages per-edge metadata
(local gather index, in-tile dst value, cj value) in slot order.
"""
import sys, os
sys.path.insert(0, '/opt/trn_rl_repo')

import numpy as np

N_NODES = 100000
OUT_DIM = 64
N_CORES = 8
DST_PER_CORE = N_NODES // N_CORES          # 12500
N_CHUNKS = 4                                # int16 idx -> <=32767 rows per chunk
CHUNK = N_NODES // N_CHUNKS                 # 25000
PAD_SENTINEL = 999.0


def _round_up(x, m):
    return (x + m - 1) // m * m


def _host_prep(src, dst, cj):
    """Partition edges by dst range, sort by (dst_tile, src_chunk, src), build
    per-core slot-packed gather indices, in-tile dst values and cj values,
    plus the shared static envelope table E[tile][chunk] (max over cores,
    rounded up to 128)."""
    n_tiles = _round_up(DST_PER_CORE, 128) // 128       # 98
    per_core = []
    counts = np.zeros((N_CORES, n_tiles, N_CHUNKS), np.int64)
    for k in range(N_CORES):
        m = (dst // DST_PER_CORE) == k
        s = src[m]
        dl = dst[m] - k * DST_PER_CORE
        t = dl // 128
        c = s // CHUNK
        order = np.lexsort((s, c, t))
        s, dl, t, c = s[order], dl[order], t[order], c[order]
        seg = t * N_CHUNKS + c
        counts[k] = np.bincount(seg, minlength=n_tiles * N_CHUNKS).reshape(
            n_tiles, N_CHUNKS)
        per_core.append((s, dl, t, c, seg))

    env = _round_up(counts.max(axis=0), 128).astype(np.int64)   # [n_tiles, N_CHUNKS]
    env_flat = env.reshape(-1)
    seg_off = np.concatenate([[0], np.cumsum(env_flat)])[:-1]
    total = int(env_flat.sum())

    idx_all, dv_all, cjv_all = [], [], []
    for k in range(N_CORES):
        s, dl, t, c, seg = per_core[k]
        seg_counts = np.bincount(seg, minlength=n_tiles * N_CHUNKS)
        within = np.arange(len(s)) - np.repeat(
            np.concatenate([[0], np.cumsum(seg_counts)])[:-1], seg_counts)
        slot = seg_off[seg] + within
        idx_flat = np.zeros(total, np.int16)
        idx_flat[slot] = (s - c * CHUNK).astype(np.int16)
        dv_flat = np.full(total, PAD_SENTINEL, np.float32)
        dv_flat[slot] = (dl - t * 128).astype(np.float32)
        cjv_flat = np.zeros(total, np.float32)
        cjv_flat[slot] = cj[s]
        # wrap idx into 16 partitions, replicate x8 (one copy per Q7 core)
        idx_all.append(np.tile(idx_flat.reshape(total // 16, 16).T, (8, 1)).copy())
        dv_all.append(dv_flat.reshape(total // 128, 128).T.copy())
        cjv_all.append(cjv_flat.reshape(total // 128, 128).T.copy())
    return env, seg_off, total, idx_all, dv_all, cjv_all


def _build_program(env, seg_off, total):
    import concourse.bass as bass
    import concourse.bacc as bacc
    import concourse.mybir as mybir
    import concourse.tile as tile

    n_tiles = env.shape[0]
    dst_pad = n_tiles * 128                              # 12544
    f32 = mybir.dt.float32

    nc = bacc.Bacc("TRN2", target_bir_lowering=False, debug=False,
                   num_devices=N_CORES)
    w_d = nc.dram_tensor("w", [N_NODES, OUT_DIM], f32, kind="ExternalInput").ap()
    ci_d = nc.dram_tensor("ci", [128, n_tiles], f32, kind="ExternalInput").ap()
    iota_d = nc.dram_tensor("iota", [128, 128], f32, kind="ExternalInput").ap()
    idx_d = nc.dram_tensor("idx", [128, total // 16], mybir.dt.int16,
                           kind="ExternalInput").ap()
    dv_d = nc.dram_tensor("dv", [128, total // 128], f32,
                          kind="ExternalInput").ap()
    cjv_d = nc.dram_tensor("cjv", [128, total // 128], f32,
                           kind="ExternalInput").ap()
    out_d = nc.dram_tensor("out", [dst_pad, OUT_DIM], f32,
                           kind="ExternalOutput").ap()

    out_v = out_d.rearrange("(n p) d -> n p d", p=128)   # [98, 128, 64]

    blocks = env.sum(axis=1) // 128                      # per-tile block count
    max_blocks = int(blocks.max())

    with tile.TileContext(nc) as tc:
        with (
            tc.tile_pool(name="const", bufs=1) as constp,
            tc.tile_pool(name="msg", bufs=3) as msgp,
            tc.tile_pool(name="oh", bufs=6) as ohp,
            tc.tile_pool(name="ps", bufs=2, space="PSUM") as psp,
            tc.tile_pool(name="ot", bufs=3) as otp,
        ):
            ci_t = constp.tile([128, n_tiles], f32)
            io_t = constp.tile([128, 128], f32)
            idx_t = constp.tile([128, total // 16], mybir.dt.int16)
            dv_t = constp.tile([128, total // 128], f32)
            cjv_t = constp.tile([128, total // 128], f32)
            nc.sync.dma_start(ci_t[:], ci_d[:])
            nc.sync.dma_start(io_t[:], iota_d[:])
            nc.sync.dma_start(idx_t[:], idx_d[:])
            nc.sync.dma_start(dv_t[:], dv_d[:])
            nc.sync.dma_start(cjv_t[:], cjv_d[:])

            for t in range(n_tiles):
                n_blk = int(blocks[t])
                msg = msgp.tile([128, max_blocks, OUT_DIM], f32, tag="msg")
                col = 0
                for c in range(N_CHUNKS):
                    e_tc = int(env[t, c])
                    if e_tc == 0:
                        continue
                    off = int(seg_off[t * N_CHUNKS + c])
                    # single_packet packs each engine's descriptors into one
                    # packet; packets are HW-capped at 64 descs, so gathers
                    # over 1024 idxs (64*16 engines) wedge the device.
                    nc.gpsimd.dma_gather(
                        msg[:, col:col + e_tc // 128, :],
                        w_d[c * CHUNK:c * CHUNK + CHUNK, :],
                        idx_t[:, off // 16:(off + e_tc) // 16],
                        e_tc, e_tc, OUT_DIM,
                        single_packet=(e_tc <= 1024),
                    )
                    col += e_tc // 128
                ps = psp.tile([128, OUT_DIM], f32)
                g0 = int(seg_off[t * N_CHUNKS]) // 128
                for b in range(n_blk):
                    oh = ohp.tile([128, 128], f32, tag="oh")
                    # oh[p, d] = (iota[d] == dstval[p]) * cj[p]
                    nc.vector.tensor_scalar(
                        oh[:], io_t[:], dv_t[:, g0 + b:g0 + b + 1],
                        cjv_t[:, g0 + b:g0 + b + 1],
                        mybir.AluOpType.is_equal, mybir.AluOpType.mult)
                    nc.tensor.matmul(ps[:], oh[:], msg[:, b, :],
                                     start=(b == 0), stop=(b == n_blk - 1))
                ot = otp.tile([128, OUT_DIM], f32, tag="ot")
                nc.scalar.activation(ot[:], ps[:],
                                     mybir.ActivationFunctionType.Copy,
                                     scale=ci_t[:, t:t + 1])
                nc.sync.dma_start(out_v[t], ot[:])

    nc.compile()
    return nc


def prepare(node_ids, src_idx, dst_idx, cj, ci, weight):
    """Host prep + program build. Returns (nc, in_maps, postprocess)."""
    import time
    _t0 = time.time()

    node_ids = np.asarray(node_ids)
    src = np.asarray(src_idx).astype(np.int64)
    dst = np.asarray(dst_idx).astype(np.int64)
    cj = np.asarray(cj, dtype=np.float32).reshape(-1)
    ci = np.asarray(ci, dtype=np.float32).reshape(-1)
    weight = np.ascontiguousarray(np.asarray(weight, dtype=np.float32))

    # feat rows are weight[node_ids]; with the arange fill this is identity
    if not np.array_equal(node_ids, np.arange(N_NODES, dtype=node_ids.dtype)):
        weight = np.ascontiguousarray(weight[node_ids])

    n_tiles = _round_up(DST_PER_CORE, 128) // 128
    iota = np.tile(np.arange(128, dtype=np.float32), (128, 1))

    env, seg_off, total, idx_all, dv_all, cjv_all = _host_prep(src, dst, cj)
    print(f"[kernel] host prep: {time.time()-_t0:.1f}s (total slots {total})",
          flush=True)
    _t1 = time.time()
    nc = _build_program(env, seg_off, total)
    print(f"[kernel] build+schedule+compile-to-bir: {time.time()-_t1:.1f}s",
          flush=True)

    in_maps = []
    for k in range(N_CORES):
        ci_k = np.zeros(n_tiles * 128, np.float32)
        ci_k[:DST_PER_CORE] = ci[k * DST_PER_CORE:(k + 1) * DST_PER_CORE]
        ci_w = ci_k.reshape(n_tiles, 128).T.copy()
        in_maps.append({
            "w": weight, "ci": ci_w, "iota": iota,
            "idx": idx_all[k], "dv": dv_all[k], "cjv": cjv_all[k],
        })

    def post(results):
        return np.concatenate(
            [results[k]["out"][:DST_PER_CORE] for k in range(N_CORES)], axis=0)

    return nc, in_maps, post


def kernel(node_ids, src_idx, dst_idx, cj, ci, weight):
    import time
    from concourse.bass_utils import run_bass_kernel_spmd
    nc, in_maps, post = prepare(node_ids, src_idx, dst_idx, cj, ci, weight)
    _t2 = time.time()
    res = run_bass_kernel_spmd(nc, in_maps, core_ids=list(range(N_CORES)))
    print(f"[kernel] neff compile+exec: {time.time()-_t2:.1f}s", flush=True)
    return post(res.results)


# revision 9
# speedup vs baseline: 14.5018x; 14.5018x over previous
"""GCMC graph-conv kernel for Trainium2, 8-core SPMD.

out = ci * segment_sum((weight[node_ids] * cj)[src_idx], dst_idx)

Strategy (edge sharding by dst range):
  - core k owns dst rows [k*12500, (k+1)*12500); its edges are host-partitioned
    and sorted by (dst_tile, src_chunk, src)
  - messages are fetched straight from the weight table with SWDGE dma_gather
    (int16 indices -> the table is addressed in 4 chunks of 25000 rows)
  - the cj scale is folded into the segment-sum matmul: for each block of 128
    gathered rows, DVE builds onehot[p, d] = (iota[d] == dst_local[p]) * cj[p]
    in one tensor_scalar op, and TensorE accumulates psum[d, :] += onehot.T @ w
  - flush: ACT copies psum*ci to SBUF, HWDGE DMAs the dst tile out

All multiplies stay on device (cj lands in the one-hot, applied by the PE);
the host only partitions/sorts edges and stages per-edge metadata
(local gather index, in-tile dst value, cj value) in slot order.
"""
import sys, os
sys.path.insert(0, '/opt/trn_rl_repo')

import numpy as np

N_NODES = 100000
OUT_DIM = 64
N_CORES = 8
DST_PER_CORE = N_NODES // N_CORES          # 12500
N_CHUNKS = 4                                # int16 idx -> <=32767 rows per chunk
CHUNK = N_NODES // N_CHUNKS                 # 25000
PAD_SENTINEL = 999.0


def _round_up(x, m):
    return (x + m - 1) // m * m


def _host_prep(src, dst, cj):
    """Partition edges by dst range, sort by (dst_tile, src_chunk, src), build
    per-core slot-packed gather indices, in-tile dst values and cj values,
    plus the shared static envelope table E[tile][chunk] (max over cores,
    rounded up to 128)."""
    n_tiles = _round_up(DST_PER_CORE, 128) // 128       # 98
    per_core = []
    counts = np.zeros((N_CORES, n_tiles, N_CHUNKS), np.int64)
    for k in range(N_CORES):
        m = (dst // DST_PER_CORE) == k
        s = src[m]
        dl = dst[m] - k * DST_PER_CORE
        t = dl // 128
        c = s // CHUNK
        order = np.lexsort((s, c, t))
        s, dl, t, c = s[order], dl[order], t[order], c[order]
        seg = t * N_CHUNKS + c
        counts[k] = np.bincount(seg, minlength=n_tiles * N_CHUNKS).reshape(
            n_tiles, N_CHUNKS)
        per_core.append((s, dl, t, c, seg))

    env = _round_up(counts.max(axis=0), 128).astype(np.int64)   # [n_tiles, N_CHUNKS]
    env_flat = env.reshape(-1)
    seg_off = np.concatenate([[0], np.cumsum(env_flat)])[:-1]
    total = int(env_flat.sum())

    idx_all, dv_all, cjv_all = [], [], []
    for k in range(N_CORES):
        s, dl, t, c, seg = per_core[k]
        seg_counts = np.bincount(seg, minlength=n_tiles * N_CHUNKS)
        within = np.arange(len(s)) - np.repeat(
            np.concatenate([[0], np.cumsum(seg_counts)])[:-1], seg_counts)
        slot = seg_off[seg] + within
        idx_flat = np.zeros(total, np.int16)
        idx_flat[slot] = (s - c * CHUNK).astype(np.int16)
        dv_flat = np.full(total, PAD_SENTINEL, np.float32)
        dv_flat[slot] = (dl - t * 128).astype(np.float32)
        cjv_flat = np.zeros(total, np.float32)
        cjv_flat[slot] = cj[s]
        # wrap idx into 16 partitions, replicate x8 (one copy per Q7 core)
        idx_all.append(np.tile(idx_flat.reshape(total // 16, 16).T, (8, 1)).copy())
        dv_all.append(dv_flat.reshape(total // 128, 128).T.copy())
        cjv_all.append(cjv_flat.reshape(total // 128, 128).T.copy())
    return env, seg_off, total, idx_all, dv_all, cjv_all


def _build_program(env, seg_off, total):
    import concourse.bass as bass
    import concourse.bacc as bacc
    import concourse.mybir as mybir
    import concourse.tile as tile

    n_tiles = env.shape[0]
    dst_pad = n_tiles * 128                              # 12544
    f32 = mybir.dt.float32

    nc = bacc.Bacc("TRN2", target_bir_lowering=False, debug=False,
                   num_devices=N_CORES)
    w_d = nc.dram_tensor("w", [N_NODES, OUT_DIM], f32, kind="ExternalInput").ap()
    ci_d = nc.dram_tensor("ci", [128, n_tiles], f32, kind="ExternalInput").ap()
    iota_d = nc.dram_tensor("iota", [128, 128], f32, kind="ExternalInput").ap()
    idx_d = nc.dram_tensor("idx", [128, total // 16], mybir.dt.int16,
                           kind="ExternalInput").ap()
    dv_d = nc.dram_tensor("dv", [128, total // 128], f32,
                          kind="ExternalInput").ap()
    cjv_d = nc.dram_tensor("cjv", [128, total // 128], f32,
                           kind="ExternalInput").ap()
    out_d = nc.dram_tensor("out", [dst_pad, OUT_DIM], f32,
                           kind="ExternalOutput").ap()

    out_v = out_d.rearrange("(n p) d -> n p d", p=128)   # [98, 128, 64]

    blocks = env.sum(axis=1) // 128                      # per-tile block count
    max_blocks = int(blocks.max())
    # every dst tile must see at least one block: an empty tile would leave
    # its PSUM/output unwritten (cannot happen with 3.2M uniform edges)
    assert (blocks > 0).all()

    with tile.TileContext(nc) as tc:
        with (
            tc.tile_pool(name="const", bufs=1) as constp,
            tc.tile_pool(name="msg", bufs=3) as msgp,
            tc.tile_pool(name="oh", bufs=6) as ohp,
            tc.tile_pool(name="ps", bufs=2, space="PSUM") as psp,
            tc.tile_pool(name="ot", bufs=3) as otp,
        ):
            ci_t = constp.tile([128, n_tiles], f32)
            io_t = constp.tile([128, 128], f32)
            idx_t = constp.tile([128, total // 16], mybir.dt.int16)
            dv_t = constp.tile([128, total // 128], f32)
            cjv_t = constp.tile([128, total // 128], f32)
            nc.sync.dma_start(ci_t[:], ci_d[:])
            nc.sync.dma_start(io_t[:], iota_d[:])
            nc.sync.dma_start(idx_t[:], idx_d[:])
            nc.sync.dma_start(dv_t[:], dv_d[:])
            nc.sync.dma_start(cjv_t[:], cjv_d[:])

            for t in range(n_tiles):
                n_blk = int(blocks[t])
                msg = msgp.tile([128, max_blocks, OUT_DIM], f32, tag="msg")
                col = 0
                for c in range(N_CHUNKS):
                    e_tc = int(env[t, c])
                    if e_tc == 0:
                        continue
                    off = int(seg_off[t * N_CHUNKS + c])
                    # single_packet packs each engine's descriptors into one
                    # packet; packets are HW-capped at 64 descs, so gathers
                    # over 1024 idxs (64*16 engines) wedge the device.
                    nc.gpsimd.dma_gather(
                        msg[:, col:col + e_tc // 128, :],
                        w_d[c * CHUNK:c * CHUNK + CHUNK, :],
                        idx_t[:, off // 16:(off + e_tc) // 16],
                        e_tc, e_tc, OUT_DIM,
                        single_packet=(e_tc <= 1024),
                    )
                    col += e_tc // 128
                ps = psp.tile([128, OUT_DIM], f32)
                g0 = int(seg_off[t * N_CHUNKS]) // 128
                for b in range(n_blk):
                    oh = ohp.tile([128, 128], f32, tag="oh")
                    # oh[p, d] = (iota[d] == dstval[p]) * cj[p]
                    nc.vector.tensor_scalar(
                        oh[:], io_t[:], dv_t[:, g0 + b:g0 + b + 1],
                        cjv_t[:, g0 + b:g0 + b + 1],
                        mybir.AluOpType.is_equal, mybir.AluOpType.mult)
                    nc.tensor.matmul(ps[:], oh[:], msg[:, b, :],
                                     start=(b == 0), stop=(b == n_blk - 1))
                ot = otp.tile([128, OUT_DIM], f32, tag="ot")
                nc.scalar.activation(ot[:], ps[:],
                                     mybir.ActivationFunctionType.Copy,
                                     scale=ci_t[:, t:t + 1])
                nc.sync.dma_start(out_v[t], ot[:])

    nc.compile()
    return nc


def prepare(node_ids, src_idx, dst_idx, cj, ci, weight):
    """Host prep + program build. Returns (nc, in_maps, postprocess)."""
    import time
    _t0 = time.time()

    node_ids = np.asarray(node_ids)
    src = np.asarray(src_idx).astype(np.int64)
    dst = np.asarray(dst_idx).astype(np.int64)
    cj = np.asarray(cj, dtype=np.float32).reshape(-1)
    ci = np.asarray(ci, dtype=np.float32).reshape(-1)
    weight = np.ascontiguousarray(np.asarray(weight, dtype=np.float32))

    # feat rows are weight[node_ids]; with the arange fill this is identity
    if not np.array_equal(node_ids, np.arange(N_NODES, dtype=node_ids.dtype)):
        weight = np.ascontiguousarray(weight[node_ids])

    n_tiles = _round_up(DST_PER_CORE, 128) // 128
    iota = np.tile(np.arange(128, dtype=np.float32), (128, 1))

    env, seg_off, total, idx_all, dv_all, cjv_all = _host_prep(src, dst, cj)
    print(f"[kernel] host prep: {time.time()-_t0:.1f}s (total slots {total})",
          flush=True)
    _t1 = time.time()
    nc = _build_program(env, seg_off, total)
    print(f"[kernel] build+schedule+compile-to-bir: {time.time()-_t1:.1f}s",
          flush=True)

    in_maps = []
    for k in range(N_CORES):
        ci_k = np.zeros(n_tiles * 128, np.float32)
        ci_k[:DST_PER_CORE] = ci[k * DST_PER_CORE:(k + 1) * DST_PER_CORE]
        ci_w = ci_k.reshape(n_tiles, 128).T.copy()
        in_maps.append({
            "w": weight, "ci": ci_w, "iota": iota,
            "idx": idx_all[k], "dv": dv_all[k], "cjv": cjv_all[k],
        })

    def post(results):
        return np.concatenate(
            [results[k]["out"][:DST_PER_CORE] for k in range(N_CORES)], axis=0)

    return nc, in_maps, post


def kernel(node_ids, src_idx, dst_idx, cj, ci, weight):
    import time
    from concourse.bass_utils import run_bass_kernel_spmd
    nc, in_maps, post = prepare(node_ids, src_idx, dst_idx, cj, ci, weight)
    _t2 = time.time()
    res = run_bass_kernel_spmd(nc, in_maps, core_ids=list(range(N_CORES)))
    print(f"[kernel] neff compile+exec: {time.time()-_t2:.1f}s", flush=True)
    return post(res.results)
